# revision 2
# baseline (speedup 1.0000x reference)
"""Bass/Tile TRN2 kernel for nn_AttentionHead (B=64, N=1024, d=512), 8-core data parallel.

Math (per batch):
    proj  = x @ W1 + b1                      [N, 2d]
    S     = proj @ relu(proj).T / sqrt(2d)   [N, N]
    P     = softmax(S, axis=-1)
    F     = P @ proj                         [N, 2d]
    out   = relu(F @ W2 + b2)                [N, d]

Kernel dataflow (transposed-score formulation, avoids transposing P):
    xT    = x.T (PE transpose)                                  [d, N]
    projT = W1.T @ xT + b1; keyT = relu(projT)                  [2d, N]
    St    = keyT.T-contraction:  St[m,n] = sum_e keyT[e,m] projT[e,n]
    Et    = exp(St / 32)                                        [m, n]
    r[n]  = sum_m Et[m,n]            (ones-column matmul)
    G     = x.T-contraction: G[d,n] = sum_m x[m,d] Et[m,n]
    H     = W1-contraction:  H[e,n] = sum_d W1[d,e] G[d,n]
          (= unnormalized P@​(x@W1) transposed; b1's contribution through the
           value path is r[n]*b1[e], folded into the fc2 bias row c below)
    Z     = H-contraction:   Z[n,t] = sum_e H[e,n] W2[e,t] + r[n]*c[t],
            c = b1 @ W2 + b2   (rank-1 matmul appends the bias terms)
    out   = relu(Z[n,t] / r[n])
All matmul operands bf16 (fp32 PSUM accumulate).
"""

import numpy as np

B, N, D = 64, 1024, 512
E = 2 * D
NCORES = 8
BPC = B // NCORES
P = 128
MG = N // P  # 8 token groups
DG = D // P  # 4 d groups
EG = E // P  # 8 e groups
NJ = N // 512  # 2 free-dim chunks
SCALE = float(1.0 / np.sqrt(2.0 * D))

_CACHE = {}


def _build(bpc=BPC):
    import concourse.mybir as mybir
    import concourse.tile as tile
    from concourse import bacc
    from concourse.masks import make_identity
    from contextlib import ExitStack

    BF = mybir.dt.bfloat16
    F32 = mybir.dt.float32
    AF = mybir.ActivationFunctionType
    ALU = mybir.AluOpType

    nc = bacc.Bacc("TRN2", target_bir_lowering=False, debug=False, num_devices=NCORES)
    x_d = nc.dram_tensor("x", [bpc, N, D], F32, kind="ExternalInput").ap()
    w1_d = nc.dram_tensor("W1", [D, E], F32, kind="ExternalInput").ap()
    b1_d = nc.dram_tensor("bias1", [E], F32, kind="ExternalInput").ap()
    w2_d = nc.dram_tensor("W2", [E, D], F32, kind="ExternalInput").ap()
    b2_d = nc.dram_tensor("bias2", [D], F32, kind="ExternalInput").ap()
    out_d = nc.dram_tensor("out", [bpc, N, D], F32, kind="ExternalOutput").ap()

    with tile.TileContext(nc) as tc, ExitStack() as ctx:
        stage = ctx.enter_context(tc.tile_pool(name="stage", bufs=2))
        consts = ctx.enter_context(tc.tile_pool(name="consts", bufs=1))
        xbf_p = ctx.enter_context(tc.tile_pool(name="xbf", bufs=2))
        xt_p = ctx.enter_context(tc.tile_pool(name="xt", bufs=1))
        projT_p = ctx.enter_context(tc.tile_pool(name="projT", bufs=1))
        keyT_p = ctx.enter_context(tc.tile_pool(name="keyT", bufs=1))
        e_p = ctx.enter_context(tc.tile_pool(name="Et", bufs=1))
        g_p = ctx.enter_context(tc.tile_pool(name="Gt", bufs=1))
        h_p = ctx.enter_context(tc.tile_pool(name="Ht", bufs=1))
        outp = ctx.enter_context(tc.tile_pool(name="outp", bufs=2))
        small = ctx.enter_context(tc.tile_pool(name="small", bufs=2))
        dram = ctx.enter_context(tc.tile_pool(name="dram", bufs=2, space="DRAM"))
        psA = ctx.enter_context(tc.tile_pool(name="psA", bufs=2, space="PSUM"))
        psB = ctx.enter_context(tc.tile_pool(name="psB", bufs=2, space="PSUM"))
        psC = ctx.enter_context(tc.tile_pool(name="psC", bufs=2, space="PSUM"))

        # ---------------- constants / weights ----------------
        w1_st = stage.tile([P, DG, E], F32, tag="stage")
        nc.sync.dma_start(out=w1_st[:], in_=w1_d.rearrange("(dg p) e -> p dg e", p=P))
        w1_bf = consts.tile([P, DG, E], BF)
        nc.vector.tensor_copy(w1_bf[:], w1_st[:])

        w2_st = stage.tile([P, EG, D], F32, tag="stage")
        nc.sync.dma_start(out=w2_st[:], in_=w2_d.rearrange("(eg p) t -> p eg t", p=P))
        w2_bf = consts.tile([P, EG, D], BF)
        nc.vector.tensor_copy(w2_bf[:], w2_st[:])

        b1t = consts.tile([P, EG], F32)
        nc.sync.dma_start(out=b1t[:], in_=b1_d.rearrange("(g p) -> p g", p=P))
        b1t_bf = consts.tile([P, EG], BF)
        nc.vector.tensor_copy(b1t_bf[:], b1t[:])
        b2row = consts.tile([1, D], F32)
        nc.sync.dma_start(out=b2row[:], in_=b2_d.rearrange("(o t) -> o t", o=1))
        ones_bf = consts.tile([P, 1], BF)
        nc.vector.memset(ones_bf[:], 1.0)
        ident = consts.tile([P, P], BF)
        make_identity(nc, ident[:])

        # c = b1 @ W2 + b2   [1, D]
        ps_c = psC.tile([1, D], F32, tag="psC")
        for eg in range(EG):
            nc.tensor.matmul(
                ps_c[:], b1t_bf[:, eg : eg + 1], w2_bf[:, eg, :],
                start=(eg == 0), stop=(eg == EG - 1),
            )
        c_bf = consts.tile([1, D], BF)
        nc.vector.tensor_add(c_bf[:], ps_c[:], b2row[:])

        # ---------------- per-batch pipeline ----------------
        for b in range(bpc):
            x_st = stage.tile([P, MG, D], F32, tag="stage")
            nc.sync.dma_start(out=x_st[:], in_=x_d[b].rearrange("(g p) d -> p g d", p=P))
            x_bf = xbf_p.tile([P, MG, D], BF)
            nc.vector.tensor_copy(x_bf[:], x_st[:])

            # xT[d, m]
            xT = xt_p.tile([P, DG, N], BF)
            for mg in range(MG):
                for dg in range(DG):
                    pt = psA.tile([P, P], BF, tag="psA")
                    nc.tensor.transpose(pt[:], x_bf[:, mg, dg * P : (dg + 1) * P], ident[:])
                    nc.vector.tensor_copy(xT[:, dg, mg * P : (mg + 1) * P], pt[:])

            # fc1: projT = W1.T @ xT + b1 ; keyT = relu(projT)
            projT = projT_p.tile([P, EG, N], BF)
            keyT = keyT_p.tile([P, EG, N], BF)
            for eg in range(EG):
                for nj in range(NJ):
                    pf = psA.tile([P, 512], F32, tag="psA")
                    for dg in range(DG):
                        nc.tensor.matmul(
                            pf[:],
                            w1_bf[:, dg, eg * P : (eg + 1) * P],
                            xT[:, dg, nj * 512 : (nj + 1) * 512],
                            start=(dg == 0), stop=(dg == DG - 1),
                        )
                    nsl = slice(nj * 512, (nj + 1) * 512)
                    nc.scalar.activation(
                        projT[:, eg, nsl], pf[:], AF.Identity,
                        bias=b1t[:, eg : eg + 1], scale=1.0,
                    )
                    nc.vector.tensor_scalar(
                        out=keyT[:, eg, nsl], in0=pf[:],
                        scalar1=b1t[:, eg : eg + 1], scalar2=0.0,
                        op0=ALU.add, op1=ALU.max,
                    )

            # St[m,n] = sum_e keyT[e,m] * projT[e,n];  Et = exp(St/32)
            Et = e_p.tile([P, MG, N], BF)
            for mg in range(MG):
                for nj in range(NJ):
                    psT = psB.tile([P, 512], F32, tag="psB")
                    for eg in range(EG):
                        nc.tensor.matmul(
                            psT[:],
                            keyT[:, eg, mg * P : (mg + 1) * P],
                            projT[:, eg, nj * 512 : (nj + 1) * 512],
                            start=(eg == 0), stop=(eg == EG - 1),
                        )
                    nc.scalar.activation(
                        Et[:, mg, nj * 512 : (nj + 1) * 512], psT[:], AF.Exp,
                        bias=0.0, scale=SCALE,
                    )

            # rowsum r[n] = sum_m Et[m,n]
            r_f32 = small.tile([1, N], F32)
            r_bf = small.tile([1, N], BF)
            for nj in range(NJ):
                pr = psC.tile([1, 512], F32, tag="psC")
                for mg in range(MG):
                    nc.tensor.matmul(
                        pr[:], ones_bf[:], Et[:, mg, nj * 512 : (nj + 1) * 512],
                        start=(mg == 0), stop=(mg == MG - 1),
                    )
                nsl = slice(nj * 512, (nj + 1) * 512)
                nc.vector.tensor_copy(r_f32[:, nsl], pr[:])
                nc.vector.tensor_copy(r_bf[:, nsl], pr[:])

            # G[d,n] = sum_m x[m,d] Et[m,n]
            Gt = g_p.tile([P, DG, N], BF)
            for dg in range(DG):
                for nj in range(NJ):
                    pg = psA.tile([P, 512], F32, tag="psA")
                    for mg in range(MG):
                        nc.tensor.matmul(
                            pg[:],
                            x_bf[:, mg, dg * P : (dg + 1) * P],
                            Et[:, mg, nj * 512 : (nj + 1) * 512],
                            start=(mg == 0), stop=(mg == MG - 1),
                        )
                    nc.vector.tensor_copy(Gt[:, dg, nj * 512 : (nj + 1) * 512], pg[:])

            # H[e,n] = sum_d W1[d,e] G[d,n]
            Ht = h_p.tile([P, EG, N], BF)
            for eg in range(EG):
                for nj in range(NJ):
                    ph = psB.tile([P, 512], F32, tag="psB")
                    for dg in range(DG):
                        nc.tensor.matmul(
                            ph[:],
                            w1_bf[:, dg, eg * P : (eg + 1) * P],
                            Gt[:, dg, nj * 512 : (nj + 1) * 512],
                            start=(dg == 0), stop=(dg == DG - 1),
                        )
                    nc.scalar.copy(Ht[:, eg, nj * 512 : (nj + 1) * 512], ph[:])

            # 1/r in [n-partition, 1] layout (bounce through DRAM to transpose)
            r_dram = dram.tile([N], F32)
            nc.sync.dma_start(out=r_dram.rearrange("(o n) -> o n", o=1), in_=r_f32[:1, :])
            rT = small.tile([P, MG], F32)
            nc.sync.dma_start(out=rT[:], in_=r_dram.rearrange("(j p) -> p j", p=P))
            rinv = small.tile([P, MG], F32)
            nc.vector.reciprocal(rinv[:], rT[:])

            # fc2: Z[n,t] = sum_e H[e,n] W2[e,t] + r[n] c[t];  out = relu(Z/r)
            o_t = outp.tile([P, MG, D], F32)
            for ng in range(MG):
                po = psC.tile([P, D], F32, tag="psC")
                for eg in range(EG):
                    nc.tensor.matmul(
                        po[:],
                        Ht[:, eg, ng * P : (ng + 1) * P],
                        w2_bf[:, eg, :],
                        start=(eg == 0), stop=False,
                    )
                nc.tensor.matmul(
                    po[:], r_bf[:1, ng * P : (ng + 1) * P], c_bf[:1, :],
                    start=False, stop=True,
                )
                nc.scalar.activation(
                    o_t[:, ng, :], po[:], AF.Relu,
                    bias=0.0, scale=rinv[:, ng : ng + 1],
                )
            nc.sync.dma_start(out=out_d[b].rearrange("(g p) t -> p g t", p=P), in_=o_t[:])

    nc.compile()
    return nc


def get_nc(bpc=BPC):
    if bpc not in _CACHE:
        _CACHE[bpc] = _build(bpc)
    return _CACHE[bpc]


def kernel(x, W1, bias1, W2, bias2):
    from concourse.bass_utils import run_bass_kernel_spmd

    nc = get_nc()
    x = np.ascontiguousarray(x, dtype=np.float32)
    in_maps = [
        {
            "x": x[i * BPC : (i + 1) * BPC],
            "W1": np.asarray(W1, dtype=np.float32),
            "bias1": np.asarray(bias1, dtype=np.float32),
            "W2": np.asarray(W2, dtype=np.float32),
            "bias2": np.asarray(bias2, dtype=np.float32),
        }
        for i in range(NCORES)
    ]
    res = run_bass_kernel_spmd(nc, in_maps, list(range(NCORES)))
    return np.concatenate([res.results[i]["out"] for i in range(NCORES)], axis=0)


# revision 16
# speedup vs baseline: 1.3947x; 1.3947x over previous
"""Bass/Tile TRN2 kernel for nn_AttentionHead (B=64, N=1024, d=512), 8-core data parallel.

Math (per batch):
    proj  = x @ W1 + b1                      [N, 2d]
    S     = proj @ relu(proj).T / sqrt(2d)   [N, N]
    P     = softmax(S, axis=-1)
    F     = P @ proj                         [N, 2d]
    out   = relu(F @ W2 + b2)                [N, d]

Kernel dataflow (transposed-score formulation, avoids transposing P):
    xT    = x.T (DMA transpose)                                 [d, N]
    projT = W1.T @ xT + b1; keyT = relu(projT)                  [2d, N]
    St[m,n] = sum_e keyT[e,m] projT[e,n];  Et = exp(St / 32)    [m, n]
    r[n]  = sum_m Et[m,n]            (ones-column matmul)
    G[d,n] = sum_m x[m,d] Et[m,n]
    H[e,n] = sum_d W1[d,e] G[d,n]
          (= unnormalized P@(x@W1) transposed; b1's value-path contribution is
           r[n]*b1[e], folded into the fc2 bias row c below)
    Z[n,t] = sum_e H[e,n] W2[e,t] + r[n]*c[t],  c = b1 @ W2 + b2
    out   = relu(Z[n,t] / r[n])
All matmul operands bf16 (fp32 PSUM accumulate). Loops are ordered so each
stationary (lhsT) tile serves the two 512-wide free-dim chunks back-to-back;
a post-compile pass (_dedup_ldweights) elides the repeated LDWEIGHTS.
"""

import numpy as np

B, N, D = 64, 1024, 512
E = 2 * D
NCORES = 8
BPC = B // NCORES
P = 128
MG = N // P  # 8 token groups
DG = D // P  # 4 d groups
EG = E // P  # 8 e groups
NJ = N // 512  # 2 free-dim chunks
SCALE = float(1.0 / np.sqrt(2.0 * D))

_CACHE = {}
_PATCHED = False


def _dedup_ldweights(nc):
    """Delete redundant InstLdweights: consecutive PE weight-loads of the same
    SBUF region keep the PE array's stationary operand, so the repeat load is a
    no-op costing ~107ns. Only sync-free LDWs are removed (waits/updates were
    already hoisted by bacc's move_matmul_waits_to_ldweights)."""
    import concourse.mybir as mybir

    removed = 0
    for bb in nc.m.functions[0].blocks:
        last_key = None
        keep = []
        for inst in bb.instructions:
            if str(getattr(inst, "engine", "")) != "EngineType.PE":
                keep.append(inst)
                continue
            if isinstance(inst, mybir.InstLdweights):
                ap = inst.ins[0]
                key = (
                    getattr(ap, "memref", None),
                    getattr(ap, "offset", None),
                    str(getattr(ap, "ap", None)),
                    str(getattr(ap, "dtype", None)),
                    str(getattr(inst, "tile_position", None)),
                    str(getattr(inst, "is_transpose", None)),
                )
                si = inst.sync_info
                sync_free = si is None or (not si.on_wait and not si.on_update)
                if key == last_key and sync_free:
                    removed += 1
                    continue
                last_key = key
            keep.append(inst)
        bb.instructions[:] = keep
    return removed


def _build(bpc=BPC):
    import concourse.mybir as mybir
    import concourse.tile as tile
    from concourse import bacc
    from contextlib import ExitStack

    BF = mybir.dt.bfloat16
    F32 = mybir.dt.float32
    AF = mybir.ActivationFunctionType
    ALU = mybir.AluOpType

    nc = bacc.Bacc("TRN2", target_bir_lowering=False, debug=False, num_devices=NCORES)
    x_d = nc.dram_tensor("x", [bpc, N, D], F32, kind="ExternalInput").ap()
    w1_d = nc.dram_tensor("W1", [D, E], F32, kind="ExternalInput").ap()
    b1_d = nc.dram_tensor("bias1", [E], F32, kind="ExternalInput").ap()
    w2_d = nc.dram_tensor("W2", [E, D], F32, kind="ExternalInput").ap()
    b2_d = nc.dram_tensor("bias2", [D], F32, kind="ExternalInput").ap()
    c_d = nc.dram_tensor("c", [D], F32, kind="ExternalInput").ap()  # b1@W2 + b2 (host)
    out_d = nc.dram_tensor("out", [bpc, N, D], F32, kind="ExternalOutput").ap()

    with tile.TileContext(nc) as tc, ExitStack() as ctx:
        stage = ctx.enter_context(tc.tile_pool(name="stage", bufs=2))
        consts = ctx.enter_context(tc.tile_pool(name="consts", bufs=1))
        xbf_p = ctx.enter_context(tc.tile_pool(name="xbf", bufs=2))
        xt_p = ctx.enter_context(tc.tile_pool(name="xt", bufs=2))
        projT_p = ctx.enter_context(tc.tile_pool(name="projT", bufs=1))
        keyT_p = ctx.enter_context(tc.tile_pool(name="keyT", bufs=1))
        e_p = ctx.enter_context(tc.tile_pool(name="Et", bufs=1))
        g_p = ctx.enter_context(tc.tile_pool(name="Gt", bufs=1))
        h_p = ctx.enter_context(tc.tile_pool(name="Ht", bufs=1))
        outp = ctx.enter_context(tc.tile_pool(name="outp", bufs=2))
        small = ctx.enter_context(tc.tile_pool(name="small", bufs=2))
        dram = ctx.enter_context(tc.tile_pool(name="dram", bufs=2, space="DRAM"))
        ps = ctx.enter_context(tc.tile_pool(name="ps", bufs=6, space="PSUM"))
        psC = ctx.enter_context(tc.tile_pool(name="psC", bufs=2, space="PSUM"))

        # ---------------- first batch's x in flight before anything else ----
        x_st = stage.tile([P, MG, D], F32, tag="stage")
        nc.sync.dma_start(out=x_st[:], in_=x_d[0].rearrange("(g p) d -> p g d", p=P))

        # ---------------- constants / weights ----------------
        w1_st = stage.tile([P, DG, E], F32, tag="stage")
        nc.sync.dma_start(out=w1_st[:], in_=w1_d.rearrange("(dg p) e -> p dg e", p=P))
        w1_bf = consts.tile([P, DG, E], BF)
        nc.vector.tensor_copy(w1_bf[:], w1_st[:])

        w2_st = stage.tile([P, EG, D], F32, tag="stage")
        nc.sync.dma_start(out=w2_st[:], in_=w2_d.rearrange("(eg p) t -> p eg t", p=P))
        w2_bf = consts.tile([P, EG, D], BF)
        nc.vector.tensor_copy(w2_bf[:], w2_st[:])

        b1t = consts.tile([P, EG], F32)
        nc.sync.dma_start(out=b1t[:], in_=b1_d.rearrange("(g p) -> p g", p=P))
        ones_sq = consts.tile([P, P], BF)
        nc.vector.memset(ones_sq[:], 1.0)

        # rank-1 operands: A_pad row 0 carries r (written per batch),
        # B_pad row 0 carries c = b1@W2 + b2 (host-computed input); other rows 0.
        c_st = consts.tile([1, D], F32)
        nc.sync.dma_start(out=c_st[:], in_=c_d.rearrange("(o t) -> o t", o=1))
        A_pad = consts.tile([P, N], BF)
        nc.vector.memset(A_pad[:], 0.0)
        B_pad = consts.tile([P, D], BF)
        nc.vector.memset(B_pad[:], 0.0)
        nc.vector.tensor_copy(B_pad[0:1, :], c_st[0:1, :])

        # ---------------- per-batch pipeline ----------------
        for b in range(bpc):
            if b > 0:
                x_st = stage.tile([P, MG, D], F32, tag="stage")
                nc.sync.dma_start(
                    out=x_st[:], in_=x_d[b].rearrange("(g p) d -> p g d", p=P)
                )
            x_bf = xbf_p.tile([P, MG, D], BF)
            nc.vector.tensor_copy(x_bf[:], x_st[:])
            # xT[d, m] via bf16 xbar DMA transpose (SBUF -> SBUF), one per m-group
            xT = xt_p.tile([P, DG, N], BF)
            for mg in range(MG):
                nc.sync.dma_start(
                    out=xT[:, :, mg * P : (mg + 1) * P],
                    in_=x_bf[:, mg, :],
                    transpose=True,
                )

            # fc1: projT = W1.T @ xT + b1 ; keyT = relu(projT)
            projT = projT_p.tile([P, EG, N], BF)
            keyT = keyT_p.tile([P, EG, N], BF)
            for eg in range(EG):
                pf = [ps.tile([P, 512], F32, tag="ps", name=f"pf{eg}_{j}") for j in range(NJ)]
                for dg in range(DG):
                    for nj in range(NJ):
                        nc.tensor.matmul(
                            pf[nj][:],
                            w1_bf[:, dg, eg * P : (eg + 1) * P],
                            xT[:, dg, nj * 512 : (nj + 1) * 512],
                            start=(dg == 0), stop=(dg == DG - 1),
                        )
                for nj in range(NJ):
                    nsl = slice(nj * 512, (nj + 1) * 512)
                    nc.scalar.activation(
                        projT[:, eg, nsl], pf[nj][:], AF.Identity,
                        bias=b1t[:, eg : eg + 1], scale=1.0,
                    )
                    nc.vector.tensor_scalar(
                        out=keyT[:, eg, nsl], in0=pf[nj][:],
                        scalar1=b1t[:, eg : eg + 1], scalar2=0.0,
                        op0=ALU.add, op1=ALU.max,
                    )

            # St[m,n] = sum_e keyT[e,m] * projT[e,n];  Et = exp(St/32)
            Et = e_p.tile([P, MG, N], BF)
            for mg in range(MG):
                pst = [ps.tile([P, 512], F32, tag="ps", name=f"pst{mg}_{j}") for j in range(NJ)]
                for eg in range(EG):
                    for nj in range(NJ):
                        nc.tensor.matmul(
                            pst[nj][:],
                            keyT[:, eg, mg * P : (mg + 1) * P],
                            projT[:, eg, nj * 512 : (nj + 1) * 512],
                            start=(eg == 0), stop=(eg == EG - 1),
                        )
                for nj in range(NJ):
                    nc.scalar.activation(
                        Et[:, mg, nj * 512 : (nj + 1) * 512], pst[nj][:], AF.Exp,
                        bias=0.0, scale=SCALE,
                    )

            # rowsum r[n] = sum_m Et[m,n] (all-ones stationary; any psum row = sum)
            r_f32 = small.tile([1, N], F32)
            pr = [ps.tile([P, 512], F32, tag="ps", name=f"pr{j}") for j in range(NJ)]
            for mg in range(MG):
                for nj in range(NJ):
                    nc.tensor.matmul(
                        pr[nj][:], ones_sq[:], Et[:, mg, nj * 512 : (nj + 1) * 512],
                        start=(mg == 0), stop=(mg == MG - 1),
                    )
            for nj in range(NJ):
                nsl = slice(nj * 512, (nj + 1) * 512)
                nc.vector.tensor_copy(r_f32[:, nsl], pr[nj][0:1, :])
                nc.vector.tensor_copy(A_pad[0:1, nsl], pr[nj][0:1, :])

            # G[d,n] = sum_m x[m,d] Et[m,n]
            Gt = g_p.tile([P, DG, N], BF)
            for dg in range(DG):
                pg = [ps.tile([P, 512], F32, tag="ps", name=f"pg{dg}_{j}") for j in range(NJ)]
                for mg in range(MG):
                    for nj in range(NJ):
                        nc.tensor.matmul(
                            pg[nj][:],
                            x_bf[:, mg, dg * P : (dg + 1) * P],
                            Et[:, mg, nj * 512 : (nj + 1) * 512],
                            start=(mg == 0), stop=(mg == MG - 1),
                        )
                for nj in range(NJ):
                    nc.vector.tensor_copy(
                        Gt[:, dg, nj * 512 : (nj + 1) * 512], pg[nj][:]
                    )

            # H[e,n] = sum_d W1[d,e] G[d,n]
            Ht = h_p.tile([P, EG, N], BF)
            for eg in range(EG):
                ph = [ps.tile([P, 512], F32, tag="ps", name=f"ph{eg}_{j}") for j in range(NJ)]
                for dg in range(DG):
                    for nj in range(NJ):
                        nc.tensor.matmul(
                            ph[nj][:],
                            w1_bf[:, dg, eg * P : (eg + 1) * P],
                            Gt[:, dg, nj * 512 : (nj + 1) * 512],
                            start=(dg == 0), stop=(dg == DG - 1),
                        )
                for nj in range(NJ):
                    nc.scalar.copy(Ht[:, eg, nj * 512 : (nj + 1) * 512], ph[nj][:])

            # 1/r in [n-partition, 1] layout (bounce through DRAM to transpose)
            r_dram = dram.tile([N], F32)
            nc.sync.dma_start(out=r_dram.rearrange("(o n) -> o n", o=1), in_=r_f32[:1, :])
            rT = small.tile([P, MG], F32)
            nc.sync.dma_start(out=rT[:], in_=r_dram.rearrange("(j p) -> p j", p=P))
            rinv = small.tile([P, MG], F32)
            nc.vector.reciprocal(rinv[:], rT[:])

            # fc2: Z[n,t] = sum_e H[e,n] W2[e,t] + r[n] c[t];  out = relu(Z/r)
            o_t = outp.tile([P, MG, D], F32)
            for ng in range(MG):
                po = psC.tile([P, D], F32, tag="psC")
                for eg in range(EG):
                    nc.tensor.matmul(
                        po[:],
                        Ht[:, eg, ng * P : (ng + 1) * P],
                        w2_bf[:, eg, :],
                        start=(eg == 0), stop=False,
                    )
                nc.tensor.matmul(
                    po[:], A_pad[:, ng * P : (ng + 1) * P], B_pad[:, :],
                    start=False, stop=True,
                )
                nc.scalar.activation(
                    o_t[:, ng, :], po[:], AF.Relu,
                    bias=0.0, scale=rinv[:, ng : ng + 1],
                )
            nc.sync.dma_start(out=out_d[b].rearrange("(g p) t -> p g t", p=P), in_=o_t[:])

    nc.compile()
    _dedup_ldweights(nc)
    return nc


def get_nc(bpc=BPC):
    if bpc not in _CACHE:
        _CACHE[bpc] = _build(bpc)
    return _CACHE[bpc]


def kernel(x, W1, bias1, W2, bias2):
    from concourse.bass_utils import run_bass_kernel_spmd

    nc = get_nc()
    x = np.ascontiguousarray(x, dtype=np.float32)
    W1 = np.asarray(W1, dtype=np.float32)
    bias1 = np.asarray(bias1, dtype=np.float32)
    W2 = np.asarray(W2, dtype=np.float32)
    bias2 = np.asarray(bias2, dtype=np.float32)
    c = (bias1 @ W2 + bias2).astype(np.float32)
    in_maps = [
        {
            "x": x[i * BPC : (i + 1) * BPC],
            "W1": W1,
            "bias1": bias1,
            "W2": W2,
            "bias2": bias2,
            "c": c,
        }
        for i in range(NCORES)
    ]
    res = run_bass_kernel_spmd(nc, in_maps, list(range(NCORES)))
    return np.concatenate([res.results[i]["out"] for i in range(NCORES)], axis=0)


# revision 19
# speedup vs baseline: 1.4080x; 1.0095x over previous
"""Bass/Tile TRN2 kernel for nn_AttentionHead (B=64, N=1024, d=512), 8-core data parallel.

Math (per batch):
    proj  = x @ W1 + b1                      [N, 2d]
    S     = proj @ relu(proj).T / sqrt(2d)   [N, N]
    P     = softmax(S, axis=-1)
    F     = P @ proj                         [N, 2d]
    out   = relu(F @ W2 + b2)                [N, d]

Kernel dataflow (transposed-score formulation, avoids transposing P):
    xT    = x.T (DMA transpose)                                 [d, N]
    projT = W1.T @ xT + b1; keyT = relu(projT)                  [2d, N]
    St[m,n] = sum_e keyT[e,m] projT[e,n];  Et = exp(St / 32)    [m, n]
    r[n]  = sum_m Et[m,n]            (ones-column matmul)
    G[d,n] = sum_m x[m,d] Et[m,n]
    H[e,n] = sum_d W1[d,e] G[d,n]
          (= unnormalized P@(x@W1) transposed; b1's value-path contribution is
           r[n]*b1[e], folded into the fc2 bias row c below)
    Z[n,t] = sum_e H[e,n] W2[e,t] + r[n]*c[t],  c = b1 @ W2 + b2
    out   = relu(Z[n,t] / r[n])
All matmul operands bf16 (fp32 PSUM accumulate). Loops are ordered so each
stationary (lhsT) tile serves the two 512-wide free-dim chunks back-to-back;
a post-compile pass (_dedup_ldweights) elides the repeated LDWEIGHTS.
"""

import numpy as np

B, N, D = 64, 1024, 512
E = 2 * D
NCORES = 8
BPC = B // NCORES
P = 128
MG = N // P  # 8 token groups
DG = D // P  # 4 d groups
EG = E // P  # 8 e groups
NJ = N // 512  # 2 free-dim chunks
SCALE = float(1.0 / np.sqrt(2.0 * D))

_CACHE = {}
_PATCHED = False


def _dedup_ldweights(nc):
    """Delete redundant InstLdweights: consecutive PE weight-loads of the same
    SBUF region keep the PE array's stationary operand, so the repeat load is a
    no-op costing ~107ns. Only sync-free LDWs are removed (waits/updates were
    already hoisted by bacc's move_matmul_waits_to_ldweights)."""
    import concourse.mybir as mybir

    removed = 0
    for bb in nc.m.functions[0].blocks:
        last_key = None
        keep = []
        for inst in bb.instructions:
            if str(getattr(inst, "engine", "")) != "EngineType.PE":
                keep.append(inst)
                continue
            if isinstance(inst, mybir.InstLdweights):
                ap = inst.ins[0]
                key = (
                    getattr(ap, "memref", None),
                    getattr(ap, "offset", None),
                    str(getattr(ap, "ap", None)),
                    str(getattr(ap, "dtype", None)),
                    str(getattr(inst, "tile_position", None)),
                    str(getattr(inst, "is_transpose", None)),
                )
                si = inst.sync_info
                sync_free = si is None or (not si.on_wait and not si.on_update)
                if key == last_key and sync_free:
                    removed += 1
                    continue
                last_key = key
            keep.append(inst)
        bb.instructions[:] = keep
    return removed


def _build(bpc=BPC):
    import concourse.mybir as mybir
    import concourse.tile as tile
    from concourse import bacc
    from contextlib import ExitStack

    BF = mybir.dt.bfloat16
    F32 = mybir.dt.float32
    AF = mybir.ActivationFunctionType
    ALU = mybir.AluOpType

    nc = bacc.Bacc("TRN2", target_bir_lowering=False, debug=False, num_devices=NCORES)
    x_d = nc.dram_tensor("x", [bpc, N, D], F32, kind="ExternalInput").ap()
    w1_d = nc.dram_tensor("W1", [D, E], F32, kind="ExternalInput").ap()
    b1_d = nc.dram_tensor("bias1", [E], F32, kind="ExternalInput").ap()
    w2_d = nc.dram_tensor("W2", [E, D], F32, kind="ExternalInput").ap()
    b2_d = nc.dram_tensor("bias2", [D], F32, kind="ExternalInput").ap()
    c_d = nc.dram_tensor("c", [D], F32, kind="ExternalInput").ap()  # b1@W2 + b2 (host)
    out_d = nc.dram_tensor("out", [bpc, N, D], F32, kind="ExternalOutput").ap()

    with tile.TileContext(nc) as tc, ExitStack() as ctx:
        stage = ctx.enter_context(tc.tile_pool(name="stage", bufs=2))
        consts = ctx.enter_context(tc.tile_pool(name="consts", bufs=1))
        xbf_p = ctx.enter_context(tc.tile_pool(name="xbf", bufs=2))
        xt_p = ctx.enter_context(tc.tile_pool(name="xt", bufs=2))
        projT_p = ctx.enter_context(tc.tile_pool(name="projT", bufs=1))
        keyT_p = ctx.enter_context(tc.tile_pool(name="keyT", bufs=1))
        e_p = ctx.enter_context(tc.tile_pool(name="Et", bufs=1))
        g_p = ctx.enter_context(tc.tile_pool(name="Gt", bufs=1))
        h_p = ctx.enter_context(tc.tile_pool(name="Ht", bufs=1))
        outp = ctx.enter_context(tc.tile_pool(name="outp", bufs=2))
        small = ctx.enter_context(tc.tile_pool(name="small", bufs=2))
        dram = ctx.enter_context(tc.tile_pool(name="dram", bufs=2, space="DRAM"))
        ps = ctx.enter_context(tc.tile_pool(name="ps", bufs=6, space="PSUM"))
        psC = ctx.enter_context(tc.tile_pool(name="psC", bufs=2, space="PSUM"))

        # ---------------- first batch's x in flight before anything else ----
        x_st = stage.tile([P, MG, D], F32, tag="stage")
        nc.sync.dma_start(out=x_st[:], in_=x_d[0].rearrange("(g p) d -> p g d", p=P))

        # ---------------- constants / weights ----------------
        w1_st = stage.tile([P, DG, E], F32, tag="stage")
        nc.sync.dma_start(out=w1_st[:], in_=w1_d.rearrange("(dg p) e -> p dg e", p=P))
        w1_bf = consts.tile([P, DG, E], BF)
        nc.vector.tensor_copy(w1_bf[:], w1_st[:])

        w2_st = stage.tile([P, EG, D], F32, tag="stage")
        nc.sync.dma_start(out=w2_st[:], in_=w2_d.rearrange("(eg p) t -> p eg t", p=P))
        w2_bf = consts.tile([P, EG, D], BF)
        nc.vector.tensor_copy(w2_bf[:], w2_st[:])

        b1t = consts.tile([P, EG], F32)
        nc.sync.dma_start(out=b1t[:], in_=b1_d.rearrange("(g p) -> p g", p=P))
        ones_sq = consts.tile([P, P], BF)
        nc.vector.memset(ones_sq[:], 1.0)

        # c = b1@W2 + b2 (host-computed input), broadcast to all partitions:
        # the value-path bias contribution to fc2 is out += c[t] (post 1/r scale).
        import concourse.bass as bass_mod

        c_bcast = consts.tile([P, D], F32)
        c_src = c_d.rearrange("(o t) -> o t", o=1)
        c_bcast_ap = bass_mod.AP(
            tensor=c_src.tensor,
            offset=c_src.offset,
            ap=[[0, P], c_src.ap[1]],
        )
        nc.sync.dma_start(out=c_bcast[:], in_=c_bcast_ap)

        # ---------------- per-batch pipeline ----------------
        for b in range(bpc):
            if b > 0:
                x_st = stage.tile([P, MG, D], F32, tag="stage")
                nc.sync.dma_start(
                    out=x_st[:], in_=x_d[b].rearrange("(g p) d -> p g d", p=P)
                )
            x_bf = xbf_p.tile([P, MG, D], BF)
            nc.vector.tensor_copy(x_bf[:], x_st[:])
            # xT[d, m] via bf16 xbar DMA transpose (SBUF -> SBUF), one per m-group
            xT = xt_p.tile([P, DG, N], BF)
            for mg in range(MG):
                nc.sync.dma_start(
                    out=xT[:, :, mg * P : (mg + 1) * P],
                    in_=x_bf[:, mg, :],
                    transpose=True,
                )

            # fc1: projT = W1.T @ xT + b1 ; keyT = relu(projT)
            projT = projT_p.tile([P, EG, N], BF)
            keyT = keyT_p.tile([P, EG, N], BF)
            for eg in range(EG):
                pf = [ps.tile([P, 512], F32, tag="ps", name=f"pf{eg}_{j}") for j in range(NJ)]
                for dg in range(DG):
                    for nj in range(NJ):
                        nc.tensor.matmul(
                            pf[nj][:],
                            w1_bf[:, dg, eg * P : (eg + 1) * P],
                            xT[:, dg, nj * 512 : (nj + 1) * 512],
                            start=(dg == 0), stop=(dg == DG - 1),
                        )
                for nj in range(NJ):
                    nsl = slice(nj * 512, (nj + 1) * 512)
                    nc.scalar.activation(
                        projT[:, eg, nsl], pf[nj][:], AF.Identity,
                        bias=b1t[:, eg : eg + 1], scale=1.0,
                    )
                    nc.vector.tensor_scalar(
                        out=keyT[:, eg, nsl], in0=pf[nj][:],
                        scalar1=b1t[:, eg : eg + 1], scalar2=0.0,
                        op0=ALU.add, op1=ALU.max,
                    )

            # St[m,n] = sum_e keyT[e,m] * projT[e,n];  Et = exp(St/32)
            Et = e_p.tile([P, MG, N], BF)
            for mg in range(MG):
                pst = [ps.tile([P, 512], F32, tag="ps", name=f"pst{mg}_{j}") for j in range(NJ)]
                for eg in range(EG):
                    for nj in range(NJ):
                        nc.tensor.matmul(
                            pst[nj][:],
                            keyT[:, eg, mg * P : (mg + 1) * P],
                            projT[:, eg, nj * 512 : (nj + 1) * 512],
                            start=(eg == 0), stop=(eg == EG - 1),
                        )
                for nj in range(NJ):
                    nc.scalar.activation(
                        Et[:, mg, nj * 512 : (nj + 1) * 512], pst[nj][:], AF.Exp,
                        bias=0.0, scale=SCALE,
                    )

            # rowsum r[n] = sum_m Et[m,n] (all-ones stationary; any psum row = sum)
            r_f32 = small.tile([1, N], F32)
            pr = [ps.tile([P, 512], F32, tag="ps", name=f"pr{j}") for j in range(NJ)]
            for mg in range(MG):
                for nj in range(NJ):
                    nc.tensor.matmul(
                        pr[nj][:], ones_sq[:], Et[:, mg, nj * 512 : (nj + 1) * 512],
                        start=(mg == 0), stop=(mg == MG - 1),
                    )
            for nj in range(NJ):
                nsl = slice(nj * 512, (nj + 1) * 512)
                nc.vector.tensor_copy(r_f32[:, nsl], pr[nj][0:1, :])

            # G[d,n] = sum_m x[m,d] Et[m,n]
            Gt = g_p.tile([P, DG, N], BF)
            for dg in range(DG):
                pg = [ps.tile([P, 512], F32, tag="ps", name=f"pg{dg}_{j}") for j in range(NJ)]
                for mg in range(MG):
                    for nj in range(NJ):
                        nc.tensor.matmul(
                            pg[nj][:],
                            x_bf[:, mg, dg * P : (dg + 1) * P],
                            Et[:, mg, nj * 512 : (nj + 1) * 512],
                            start=(mg == 0), stop=(mg == MG - 1),
                        )
                for nj in range(NJ):
                    nc.vector.tensor_copy(
                        Gt[:, dg, nj * 512 : (nj + 1) * 512], pg[nj][:]
                    )

            # H[e,n] = sum_d W1[d,e] G[d,n]
            Ht = h_p.tile([P, EG, N], BF)
            for eg in range(EG):
                ph = [ps.tile([P, 512], F32, tag="ps", name=f"ph{eg}_{j}") for j in range(NJ)]
                for dg in range(DG):
                    for nj in range(NJ):
                        nc.tensor.matmul(
                            ph[nj][:],
                            w1_bf[:, dg, eg * P : (eg + 1) * P],
                            Gt[:, dg, nj * 512 : (nj + 1) * 512],
                            start=(dg == 0), stop=(dg == DG - 1),
                        )
                for nj in range(NJ):
                    nc.scalar.copy(Ht[:, eg, nj * 512 : (nj + 1) * 512], ph[nj][:])

            # 1/r in [n-partition, 1] layout (bounce through DRAM to transpose)
            r_dram = dram.tile([N], F32)
            nc.sync.dma_start(out=r_dram.rearrange("(o n) -> o n", o=1), in_=r_f32[:1, :])
            rT = small.tile([P, MG], F32)
            nc.sync.dma_start(out=rT[:], in_=r_dram.rearrange("(j p) -> p j", p=P))
            rinv = small.tile([P, MG], F32)
            nc.vector.reciprocal(rinv[:], rT[:])

            # fc2: Z[n,t] = sum_e H[e,n] W2[e,t];  out = relu(Z/r + c)
            o_t = outp.tile([P, MG, D], F32)
            for ng in range(MG):
                po = psC.tile([P, D], F32, tag="psC")
                for eg in range(EG):
                    nc.tensor.matmul(
                        po[:],
                        Ht[:, eg, ng * P : (ng + 1) * P],
                        w2_bf[:, eg, :],
                        start=(eg == 0), stop=(eg == EG - 1),
                    )
                osl = o_t[:, ng, :]
                nc.scalar.activation(
                    osl, po[:], AF.Copy, bias=0.0, scale=rinv[:, ng : ng + 1]
                )
                nc.vector.tensor_add(osl, osl, c_bcast[:])
                nc.vector.tensor_scalar_max(osl, osl, 0.0)
            nc.sync.dma_start(out=out_d[b].rearrange("(g p) t -> p g t", p=P), in_=o_t[:])

    nc.compile()
    _dedup_ldweights(nc)
    return nc


def get_nc(bpc=BPC):
    if bpc not in _CACHE:
        _CACHE[bpc] = _build(bpc)
    return _CACHE[bpc]


def kernel(x, W1, bias1, W2, bias2):
    from concourse.bass_utils import run_bass_kernel_spmd

    nc = get_nc()
    x = np.ascontiguousarray(x, dtype=np.float32)
    W1 = np.asarray(W1, dtype=np.float32)
    bias1 = np.asarray(bias1, dtype=np.float32)
    W2 = np.asarray(W2, dtype=np.float32)
    bias2 = np.asarray(bias2, dtype=np.float32)
    c = (bias1 @ W2 + bias2).astype(np.float32)
    in_maps = [
        {
            "x": x[i * BPC : (i + 1) * BPC],
            "W1": W1,
            "bias1": bias1,
            "W2": W2,
            "bias2": bias2,
            "c": c,
        }
        for i in range(NCORES)
    ]
    res = run_bass_kernel_spmd(nc, in_maps, list(range(NCORES)))
    return np.concatenate([res.results[i]["out"] for i in range(NCORES)], axis=0)
